# revision 7
# baseline (speedup 1.0000x reference)
"""CT-RNN cell (RK4, 6 unfolds) on 8 Trainium2 NeuronCores.

Data-parallel over batch: each core gets B/8 = 8192 rows. On-device
everything runs feature-major ([256 feat, batch] layout); the host
transposes inputs/outputs and pre-packs weight tiles.

Math (per unfold, dt = 1/6, c = dt/2):
    t_i   = tanh(v_i + b);  v_1 = z1 = q + h@W;  q = x@K
    v_i+1 = z1 + g_i * (t_i @ (g-scaled diag(s) W) / g_i ... folded)
          = z1 + (g_i*(M_i + q - v_i))     [M_i = (s*t_i) @ W]
    h'    = A*h + s*(B1 t1 + B2 t2 + B3 t3 + B4 t4)
    z1'   = q + h'@W
On device the g_i scaling is folded into pre-packed weight matrices and
identity-inject matrices, so each stage is: 4 PSUM-accumulated matmuls
(2 k-tiles of W, one q-inject, one v-inject), one DVE add (psum + z1),
one ACT tanh (bias=b per-partition).
"""

import numpy as np

import concourse.bass as bass
import concourse.tile as tile
from concourse import mybir
from concourse.bass_utils import run_bass_kernel_spmd
from concourse.vector_clock import ScopedClock

# ---------------------------------------------------------------- constants
B_FULL = 65536
D = 256
U = 256
N_CORES = 8
B_SHARD = B_FULL // N_CORES          # 8192
NUM_UNFOLDS = 6
TAU = 1.0
ELAPSED = 1.0
DT = ELAPSED / NUM_UNFOLDS
C = DT * 0.5
N_COL = 512                          # batch columns per tile (one psum bank)
N_TILES = B_SHARD // N_COL           # 16

F32 = mybir.dt.float32
F32R = mybir.dt.float32r

# h' = A*h + s*(B1 t1 + B2 t2 + B3 t3 + B4 t4)   (TAU = 1)
A_COEF = 1.0 - (DT / 6.0) * (6 - 4 * C + 2 * C * C - DT * (1 - C + C * C))
B_COEFS = [
    (DT / 6.0) * (1 - 2 * C + 2 * C * C - DT * C * C),
    (DT / 6.0) * (2 - 2 * C + DT * C),
    (DT / 6.0) * (2 - DT),
    (DT / 6.0),
]
GAMMAS = [C, C, DT]

MAX_SYNC_WAITS = 1  # this walrus build: 1 sem-wait per instruction


class SplitDrainTileContext(tile.TileContext):
    """TileContext kept for compatibility; waits are split by _split_sync_waits."""


def _split_sync_waits(nc, max_waits=MAX_SYNC_WAITS):
    """Walrus here accepts at most `max_waits` sem-waits per instruction.
    Move overflow waits onto same-engine NoOps inserted just before."""
    n_split = 0
    for bass_bb in nc.bb_map.values():
        bb = bass_bb.bb if hasattr(bass_bb, "bb") else bass_bb
        insts = bb.instructions
        out = []
        for inst in insts:
            si = inst.sync_info
            waits = list(si.on_wait) if (si and si.on_wait) else []
            if len(waits) > max_waits:
                n_split += 1
                inst.sync_info = mybir.SyncInfo(
                    on_wait=waits[: max_waits], on_update=si.on_update
                )
                rest = waits[max_waits:]
                for i in range(0, len(rest), max_waits):
                    nop = mybir.InstNoOp(
                        name=nc.get_next_instruction_name(),
                        sync_info=mybir.SyncInfo(
                            on_wait=rest[i : i + max_waits], on_update=[]
                        ),
                        bass_nofuse=True,
                        engine=inst.engine,
                    )
                    nc.register_instruction(nop)
                    out.append(nop)
            out.append(inst)
        if len(out) != len(insts):
            insts[:] = out
    return n_split


# ---------------------------------------------------------------- host packing
def _pack_w(w):
    """[256,256] -> [128, 512] lhsT tile layout: [p, kh*256 + fh*128 + m]."""
    return np.ascontiguousarray(
        w.reshape(2, 128, 2, 128).transpose(1, 0, 2, 3).reshape(128, 512)
    ).astype(np.float32)


def _pack_idents():
    """[128, 5*128]: g*128+m columns, gamma in [C, DT, 1, -C, -DT]."""
    eye = np.eye(128, dtype=np.float32)
    return np.concatenate(
        [g * eye for g in (C, DT, 1.0, -C, -DT)], axis=1
    ).astype(np.float32)


def _pack_vec(v):
    """[256] -> [128, 2] per-partition layout per feature half."""
    return np.ascontiguousarray(v.reshape(2, 128).T).astype(np.float32)


# ---------------------------------------------------------------- device program
def _build_program(n_tiles=N_TILES):
    nc = bass.Bass(target_bir_lowering=False, debug=False)
    n_cols = n_tiles * N_COL

    xT = nc.dram_tensor("xT", [D, n_cols], F32R, kind="ExternalInput").ap()
    h0T = nc.dram_tensor("h0T", [U, n_cols], F32R, kind="ExternalInput").ap()
    wr_d = nc.dram_tensor("wr", [128, 512], F32R, kind="ExternalInput").ap()
    kw_d = nc.dram_tensor("kw", [128, 512], F32R, kind="ExternalInput").ap()
    wc_d = nc.dram_tensor("wc", [128, 512], F32R, kind="ExternalInput").ap()
    wdt_d = nc.dram_tensor("wdt", [128, 512], F32R, kind="ExternalInput").ap()
    id_d = nc.dram_tensor("idents", [128, 640], F32R, kind="ExternalInput").ap()
    b_d = nc.dram_tensor("bvec", [128, 2], F32, kind="ExternalInput").ap()
    s_d = nc.dram_tensor("svec", [128, 2], F32, kind="ExternalInput").ap()
    hT_out = nc.dram_tensor("hT", [U, n_cols], F32R, kind="ExternalOutput").ap()

    AF = mybir.ActivationFunctionType
    OP = mybir.AluOpType

    from contextlib import ExitStack

    with SplitDrainTileContext(nc) as tc, ExitStack() as ctx:
        cpool = ctx.enter_context(tc.tile_pool(name="consts", bufs=1))
        wr = cpool.tile([128, 512], F32R, tag="wr")
        kw = cpool.tile([128, 512], F32R, tag="kw")
        wc = cpool.tile([128, 512], F32R, tag="wc")
        wdt = cpool.tile([128, 512], F32R, tag="wdt")
        idn = cpool.tile([128, 640], F32R, tag="idn")
        bv = cpool.tile([128, 2], F32, tag="bv")
        sv = cpool.tile([128, 2], F32, tag="sv")
        for t, d in ((wr, wr_d), (kw, kw_d), (wc, wc_d), (wdt, wdt_d),
                     (idn, id_d), (bv, b_d), (sv, s_d)):
            nc.sync.dma_start(out=t[:], in_=d[:])

        def wsl(t, kh, fh):  # lhsT slice of a packed W tile
            o = kh * 256 + fh * 128
            return t[:, o : o + 128]

        def isl(g):  # identity-inject lhsT slice
            return idn[:, g * 128 : (g + 1) * 128]

        io_pool = ctx.enter_context(tc.tile_pool(name="io", bufs=3))
        st_pool = ctx.enter_context(tc.tile_pool(name="state", bufs=2))
        t_pool = ctx.enter_context(tc.tile_pool(name="tanh", bufs=6))
        v_pool = ctx.enter_context(tc.tile_pool(name="vbuf", bufs=3))
        u_pool = ctx.enter_context(tc.tile_pool(name="ubuf", bufs=3))
        ps_pool = ctx.enter_context(tc.tile_pool(name="psum", bufs=6, space="PSUM"))

        for j in range(n_tiles):
            col = j * N_COL

            xt = io_pool.tile([128, 2, N_COL], F32R, tag="xt")
            ht = st_pool.tile([128, 2, N_COL], F32R, tag="ht")
            for fh in range(2):
                nc.sync.dma_start(
                    out=xt[:, fh], in_=xT[fh * 128 : (fh + 1) * 128, col : col + N_COL]
                )
                nc.sync.dma_start(
                    out=ht[:, fh], in_=h0T[fh * 128 : (fh + 1) * 128, col : col + N_COL]
                )

            # q = x @ K  (feature-major)
            q = st_pool.tile([128, 2, N_COL], F32R, tag="q")
            for fh in range(2):
                pq = ps_pool.tile([128, N_COL], F32, tag="ps")
                for kh in range(2):
                    nc.tensor.matmul(
                        pq[:], wsl(kw, kh, fh), xt[:, kh],
                        start=(kh == 0), stop=(kh == 1),
                    )
                nc.vector.tensor_copy(q[:, fh], pq[:])

            # z1 = q + h0 @ W
            z1 = st_pool.tile([128, 2, N_COL], F32R, tag="z1")
            for fh in range(2):
                pz = ps_pool.tile([128, N_COL], F32, tag="ps")
                for kh in range(2):
                    nc.tensor.matmul(
                        pz[:], wsl(wr, kh, fh), ht[:, kh],
                        start=(kh == 0), stop=False,
                    )
                nc.tensor.matmul(
                    pz[:], isl(2), q[:, fh], start=False, stop=True
                )
                nc.vector.tensor_copy(z1[:, fh], pz[:])

            for uf in range(NUM_UNFOLDS):
                v = z1
                ts = []
                for i in range(4):
                    t_i = t_pool.tile([128, 2, N_COL], F32R, tag="t")
                    for fh in range(2):
                        nc.scalar.activation(
                            t_i[:, fh], v[:, fh], AF.Tanh, bias=bv[:, fh : fh + 1]
                        )
                    ts.append(t_i)
                    if i < 3:
                        wg = wc if i < 2 else wdt
                        gq = 0 if i < 2 else 1   # +gamma ident col
                        gv = 3 if i < 2 else 4   # -gamma ident col
                        vn = v_pool.tile([128, 2, N_COL], F32R, tag="v")
                        for fh in range(2):
                            pb = ps_pool.tile([128, N_COL], F32, tag="ps")
                            for kh in range(2):
                                nc.tensor.matmul(
                                    pb[:], wsl(wg, kh, fh), ts[i][:, kh],
                                    start=(kh == 0), stop=False,
                                )
                            nc.tensor.matmul(
                                pb[:], isl(gq), q[:, fh],
                                start=False, stop=False,
                            )
                            nc.tensor.matmul(
                                pb[:], isl(gv), v[:, fh],
                                start=False, stop=True,
                            )
                            nc.vector.tensor_add(vn[:, fh], pb[:], z1[:, fh])
                        v = vn

                # u = B1 t1 + B2 t2 + B3 t3 + B4 t4
                ua = u_pool.tile([128, 2, N_COL], F32, tag="u")
                for fh in range(2):
                    nc.vector.tensor_scalar_mul(ua[:, fh], ts[0][:, fh], B_COEFS[0])
                for k in range(1, 4):
                    ub = u_pool.tile([128, 2, N_COL], F32, tag="u")
                    for fh in range(2):
                        nc.vector.scalar_tensor_tensor(
                            ub[:, fh], ts[k][:, fh], B_COEFS[k], ua[:, fh],
                            op0=OP.mult, op1=OP.add,
                        )
                    ua = ub
                # h' = A*h + s*u
                us = u_pool.tile([128, 2, N_COL], F32, tag="us")
                hn = st_pool.tile([128, 2, N_COL], F32R, tag="ht")
                for fh in range(2):
                    nc.vector.tensor_scalar_mul(us[:, fh], ua[:, fh], sv[:, fh : fh + 1])
                    nc.vector.scalar_tensor_tensor(
                        hn[:, fh], ht[:, fh], A_COEF, us[:, fh],
                        op0=OP.mult, op1=OP.add,
                    )
                ht = hn

                if uf < NUM_UNFOLDS - 1:
                    zn = st_pool.tile([128, 2, N_COL], F32R, tag="z1")
                    for fh in range(2):
                        pn = ps_pool.tile([128, N_COL], F32, tag="ps")
                        for kh in range(2):
                            nc.tensor.matmul(
                                pn[:], wsl(wr, kh, fh), ht[:, kh],
                                start=(kh == 0), stop=False,
                            )
                        nc.tensor.matmul(
                            pn[:], isl(2), q[:, fh],
                            start=False, stop=True,
                        )
                        nc.vector.tensor_copy(zn[:, fh], pn[:])
                    z1 = zn

            for fh in range(2):
                nc.sync.dma_start(
                    out=hT_out[fh * 128 : (fh + 1) * 128, col : col + N_COL],
                    in_=ht[:, fh],
                )

    n = _split_sync_waits(nc)
    return nc


_PROGRAM_CACHE = {}


def _get_program(n_tiles=N_TILES):
    if n_tiles not in _PROGRAM_CACHE:
        _PROGRAM_CACHE[n_tiles] = _build_program(n_tiles)
    return _PROGRAM_CACHE[n_tiles]


def _install_ntff_hook():
    """Shim antenv.axon_hooks (absent on this image) so trace=True works."""
    import sys as _sys, types as _types, ctypes as _ct, contextlib as _cl

    if "antenv.axon_hooks" in _sys.modules:
        return
    try:
        import antenv.axon_hooks  # noqa: F401

        return
    except ImportError:
        pass
    hook = None
    try:
        lib = _ct.CDLL("/opt/axon/libaxon_pjrt.so")
        if hasattr(lib, "axon_start_nrt_profile"):
            lib.axon_start_nrt_profile.argtypes = [
                _ct.POINTER(_ct.c_int64), _ct.c_size_t,
            ]
            lib.axon_start_nrt_profile.restype = _ct.c_int64
            lib.axon_stop_nrt_profile.argtypes = [_ct.c_char_p]
            lib.axon_stop_nrt_profile.restype = _ct.c_int64

            @_cl.contextmanager
            def _hook(output_dir, device_ids):
                import jax

                jax.devices()
                if device_ids:
                    ids = (_ct.c_int64 * len(device_ids))(*device_ids)
                    rc = lib.axon_start_nrt_profile(ids, len(device_ids))
                else:
                    rc = lib.axon_start_nrt_profile(None, 0)
                if rc != 0:
                    raise RuntimeError(f"axon_start_nrt_profile rc={rc}")
                try:
                    yield
                finally:
                    n = lib.axon_stop_nrt_profile(str(output_dir).encode())
                    print(f"ntff profile: {n} file(s) -> {output_dir}")

            hook = _hook
    except OSError:
        pass
    mod = _types.ModuleType("antenv.axon_hooks")
    mod.get_axon_ntff_profile_hook = lambda: hook
    mod.set_axon_ntff_profile_hook = lambda h: None
    _sys.modules["antenv.axon_hooks"] = mod
    from concourse import bass_utils as _bu

    _bu.upload_artifacts = lambda tmpdir: str(tmpdir)


# ---------------------------------------------------------------- entry point
def kernel(x, h0, kernel, recurrent_kernel, bias, scale, _n_tiles=N_TILES,
           _trace=False, _tmpdir=None):
    x = np.asarray(x, dtype=np.float32)
    h0 = np.asarray(h0, dtype=np.float32)
    W = np.asarray(recurrent_kernel, dtype=np.float64)
    K = np.asarray(kernel, dtype=np.float64)
    b = np.asarray(bias, dtype=np.float64)
    s = np.asarray(scale, dtype=np.float64)

    wp = s[:, None] * W                      # diag(s) @ W
    shared = {
        "wr": _pack_w(W),
        "kw": _pack_w(K),
        "wc": _pack_w(C * wp),
        "wdt": _pack_w(DT * wp),
        "idents": _pack_idents(),
        "bvec": _pack_vec(b),
        "svec": _pack_vec(s),
    }

    n_cols = _n_tiles * N_COL
    bsh = n_cols
    in_maps = []
    for ci in range(N_CORES):
        lo = ci * bsh
        in_maps.append(
            dict(
                shared,
                xT=np.ascontiguousarray(x[lo : lo + bsh].T),
                h0T=np.ascontiguousarray(h0[lo : lo + bsh].T),
            )
        )

    nc = _get_program(_n_tiles)
    if _trace:
        _install_ntff_hook()
    res = run_bass_kernel_spmd(
        nc, in_maps, list(range(N_CORES)), trace=_trace, tmpdir=_tmpdir
    )
    out = np.concatenate([res.results[ci]["hT"].T for ci in range(N_CORES)], axis=0)
    if _trace:
        return out.astype(np.float32), res
    return out.astype(np.float32)
